# revision 10
# baseline (speedup 1.0000x reference)
"""Single-head causal attention on 8 TRN2 NeuronCores, data-parallel over batch.

Problem: x [512, 256, 384] f32, Wq/Wk/Wv [384, 64] f32.
  q/k/v = x @ W;  S = q k^T / sqrt(384); causal softmax; out = P v.

Sharding: batch 512 -> 64 per core.  Host pre-transposes x so each device DMA
is contiguous; weights replicated.

Device: pair-granular software pipeline built so the PE never waits: every
dependency consumed by the PE is >= 2 iterations old.
  iter i: [produce: QK(quad) on even / V(quad) on odd] PV(i-4) ST(i-2)
  - qkT [128, 4, 256] = [Wq*scale | Wk] stationary over xT chunks (quad, even
    iters).  One DVE copy to SBUF bf16; k halves move to base-0 pair tiles by
    gpsimd DIRECT2D copies (one per iteration, queued after the mask select).
  - vT computed ALREADY transposed ([t, h]) with the xT chunks stationary
    (quad, odd iters); ones column seeded once per buffer slot for rowsums.
  - ST per pair in 3 blocks of 128 cols: [tri(s0,t0) | tri(s1,t1) |
    full(s0,t1)]; s>t blocks skipped.  One exp per pair ([128, 768], scalar),
    one 3D affine_select per pair zeroes both triangles (gpsimd, first in its
    queue each iteration).
  - PV (4 iterations behind) accumulates [t, 64+rowsum]; o_ps copied to a
    quad SBUF tile in bf16 (DVE for pair a, scalar for pair b) and DMA'd out
    UNNORMALIZED; the softmax division happens on the host in f32.
"""

import numpy as np

import concourse.bacc as bacc
import concourse.bass as bass
import concourse.mybir as mybir
import concourse.tile as tile
from concourse.bass_utils import run_bass_kernel_spmd

N_CORES = 8
B, T, C, H = 512, 256, 384, 64
BPC = B // N_CORES          # 64 batches per core
PAIRS = BPC // 2            # 32 pair-iterations per core
QUADS = BPC // 4            # 16 quad-DMA groups
NCHUNK = C // 128           # 3 contraction chunks
SCALE = 1.0 / np.sqrt(C)    # note: reference scales by C**-0.5, not H**-0.5

F32 = mybir.dt.float32
BF16 = mybir.dt.bfloat16
EXP = mybir.ActivationFunctionType.Exp


def build_bass():
    nc = bacc.Bacc(None, target_bir_lowering=False, debug=False)
    x_in = nc.dram_tensor("xt", [QUADS, 128, NCHUNK, 4, T], BF16, kind="ExternalInput")
    wqk_in = nc.dram_tensor("wqk", [128, NCHUNK, 128], BF16, kind="ExternalInput")
    wv_in = nc.dram_tensor("wv", [128, NCHUNK, H], BF16, kind="ExternalInput")
    out_d = nc.dram_tensor("out", [QUADS, 128, 4, 2, H + 1], BF16, kind="ExternalOutput")

    with tile.TileContext(nc) as tc:
        with (
            tc.tile_pool(name="const", bufs=1) as const_pool,
            tc.tile_pool(name="xt", bufs=3) as xt_pool,
            tc.tile_pool(name="qk_sb", bufs=3) as qk_pool,
            tc.tile_pool(name="k_sb", bufs=4) as k_pool,
            tc.tile_pool(name="v_sb", bufs=4) as v_pool,
            tc.tile_pool(name="p_sb", bufs=3) as p_pool,
            tc.tile_pool(name="ob", bufs=2) as ob_pool,
            tc.tile_pool(name="qk_ps", bufs=1, space="PSUM") as qk_ps_pool,
            tc.tile_pool(name="v_ps", bufs=2, space="PSUM") as v_ps_pool,
            tc.tile_pool(name="st_ps", bufs=1, space="PSUM") as st_ps_pool,
            tc.tile_pool(name="o_ps", bufs=2, space="PSUM") as o_ps_pool,
        ):
            wqk = const_pool.tile([128, NCHUNK, 128], BF16)
            nc.sync.dma_start(wqk[:], wqk_in[:])
            wv = const_pool.tile([128, NCHUNK, H], BF16)
            nc.sync.dma_start(wv[:], wv_in[:])

            xts, qks, ks, vs, pss, obs = {}, {}, {}, {}, {}, {}

            for i in range(PAIRS + 4):
                # ---- produce: QK(quad) on even iters, V(quad) on odd -------
                if i % 2 == 0 and i // 2 < QUADS:
                    q = i // 2
                    if i == 0:
                        for pf in range(min(3, QUADS)):
                            xts[pf] = xt_pool.tile(
                                [128, NCHUNK, 4, T], BF16, tag="xt", name="xt"
                            )
                            nc.sync.dma_start(xts[pf][:], x_in[pf])
                    elif q + 2 < QUADS:
                        xts[q + 2] = xt_pool.tile(
                            [128, NCHUNK, 4, T], BF16, tag="xt", name="xt"
                        )
                        nc.sync.dma_start(xts[q + 2][:], x_in[q + 2])

                    qk_ps = qk_ps_pool.tile([128, 4, T], F32, tag="qk")
                    qks[q] = qk_pool.tile([128, 4, T], BF16, tag="qk", name="qk_sb")
                    for h in range(2):
                        for n in range(NCHUNK):
                            nc.tensor.matmul(
                                qk_ps[:, 2 * h : 2 * h + 2, :],
                                wqk[:, n, :],
                                xts[q][:, n, 2 * h : 2 * h + 2, :],
                                start=(n == 0),
                                stop=(n == NCHUNK - 1),
                            )
                        # per-half copy releases this half's PSUM region early
                        nc.vector.tensor_copy(
                            qks[q][:, 2 * h : 2 * h + 2, :],
                            qk_ps[:, 2 * h : 2 * h + 2, :],
                        )
                elif i % 2 == 1 and i // 2 < QUADS:
                    q = i // 2
                    # v, already transposed to [t, h]: xT chunk is stationary
                    v_ps = v_ps_pool.tile([128, 4, 2, H], F32, tag="v")
                    for jj in range(4):
                        for tb in range(2):
                            for n in range(NCHUNK):
                                nc.tensor.matmul(
                                    v_ps[:, jj, tb, :],
                                    xts[q][:, n, jj, bass.ts(tb, 128)],
                                    wv[:, n, :],
                                    start=(n == 0),
                                    stop=(n == NCHUNK - 1),
                                )
                    vs[q] = v_pool.tile([128, 4, 2, H + 1], BF16, tag="v", name="v_sb")
                    if q < 4:
                        # ones column for the PV rowsum: each pool slot keeps
                        # it forever (the copy below never touches col H)
                        nc.gpsimd.memset(vs[q][:, :, :, H : H + 1], 1.0)
                    nc.vector.tensor_copy(vs[q][:, :, :, 0:H], v_ps[:])
                    del xts[q]

                # ---- PV(i-4) + writeback (mask select is 2 iterations old) -
                w = i - 4
                if 0 <= w < PAIRS:
                    qw, hw = divmod(w, 2)
                    o_ps = o_ps_pool.tile([128, 2, 2, H + 1], F32, tag="o")
                    for jl in range(2):
                        p = pss[w]
                        v = vs[qw]
                        jj = 2 * hw + jl
                        nc.tensor.matmul(
                            o_ps[:, jl, 0, :], p[:, jl, 0, :], v[:, jj, 0, :],
                            start=True, stop=True,
                        )
                        nc.tensor.matmul(
                            o_ps[:, jl, 1, :], p[:, jl, 2, :], v[:, jj, 0, :],
                            start=True, stop=False,
                        )
                        nc.tensor.matmul(
                            o_ps[:, jl, 1, :], p[:, jl, 1, :], v[:, jj, 1, :],
                            start=False, stop=True,
                        )
                    if hw == 0:
                        obs[qw] = ob_pool.tile(
                            [128, 4, 2, H + 1], BF16, tag="ob", name="ob"
                        )
                        nc.vector.tensor_copy(obs[qw][:, 0:2], o_ps[:])
                    else:
                        nc.scalar.copy(obs[qw][:, 2:4], o_ps[:])
                        nc.sync.dma_start(out_d[qw], obs[qw][:])
                        del obs[qw], vs[qw]
                    del pss[w]

                # ---- ST(i-2) + exp + causal mask ---------------------------
                m = i - 2
                if 0 <= m < PAIRS:
                    qm, hm = divmod(m, 2)
                    st = st_ps_pool.tile([128, 2, 3, 128], F32, tag="st")
                    for jl in range(2):
                        qT = qks[qm][0:H, 2 * hm + jl]   # [64, 256]
                        kT = ks[m][:, jl]                # [64, 256]
                        nc.tensor.matmul(
                            st[:, jl, 0, :], kT[:, 0:128], qT[:, 0:128],
                            start=True, stop=True,
                        )
                        nc.tensor.matmul(
                            st[:, jl, 2, :], kT[:, 0:128], qT[:, 128:T],
                            start=True, stop=True,
                        )
                        nc.tensor.matmul(
                            st[:, jl, 1, :], kT[:, 128:T], qT[:, 128:T],
                            start=True, stop=True,
                        )
                    pss[m] = p_pool.tile([128, 2, 3, 128], BF16, tag="p", name="p_sb")
                    nc.scalar.activation(pss[m][:], st[:], EXP)
                    # zero s > t in both triangular blocks of both batches:
                    # keep where col - partition >= 0
                    nc.gpsimd.affine_select(
                        out=pss[m][:, :, 0:2, :],
                        in_=pss[m][:, :, 0:2, :],
                        compare_op=mybir.AluOpType.is_ge,
                        fill=0.0,
                        base=0,
                        pattern=[[0, 2], [0, 2], [1, 128]],
                        channel_multiplier=-1,
                    )
                    del ks[m]
                    if hm == 1:
                        del qks[qm]

                # ---- tail: one k half -> base-0 pair tile per iteration ----
                # (DGE via the scalar sequencer -> runs on the DMA queues)
                if i < PAIRS:
                    q, h = divmod(i, 2)
                    ks[i] = k_pool.tile([H, 2, T], BF16, tag="k", name="k_sb")
                    nc.scalar.dma_start(
                        ks[i][:], qks[q][H:128, 2 * h : 2 * h + 2]
                    )

    nc.finalize()
    return nc


_CACHED = {}


def _get_nc():
    if "nc" not in _CACHED:
        _CACHED["nc"] = build_bass()
    return _CACHED["nc"]


def prep_inputs(x, Wq, Wk, Wv):
    import ml_dtypes

    bf16 = ml_dtypes.bfloat16
    x = np.ascontiguousarray(x, dtype=np.float32)
    wqk = np.concatenate([Wq * SCALE, Wk], axis=1).astype(np.float32)  # [384, 128]
    wqk_t = np.ascontiguousarray(
        wqk.reshape(NCHUNK, 128, 128).transpose(1, 0, 2).astype(bf16)
    )
    wv_t = np.ascontiguousarray(
        Wv.astype(np.float32).reshape(NCHUNK, 128, H).transpose(1, 0, 2).astype(bf16)
    )

    in_maps = []
    for c in range(N_CORES):
        xs = x[c * BPC : (c + 1) * BPC]  # [64, 256, 384]
        # [q, jj, t, n, p] -> [q, p, n, jj, t]  (partition-major for the DMA)
        xt = np.ascontiguousarray(
            xs.reshape(QUADS, 4, T, NCHUNK, 128).transpose(0, 4, 3, 1, 2).astype(bf16)
        )
        in_maps.append({"xt": xt, "wqk": wqk_t, "wv": wv_t})
    return in_maps


def postprocess(results):
    outs = []
    for c in range(N_CORES):
        od = results[c]["out"].astype(np.float32)  # [QUADS, 128p, 4jj, 2n, H+1]
        o = od[..., 0:H] / od[..., H : H + 1]
        outs.append(o.transpose(0, 2, 3, 1, 4).reshape(BPC, T, H))
    return np.concatenate(outs, axis=0).astype(np.float32)


def kernel(x, Wq, Wk, Wv):
    in_maps = prep_inputs(x, Wq, Wk, Wv)
    res = run_bass_kernel_spmd(_get_nc(), in_maps, core_ids=list(range(N_CORES)))
    return postprocess(res.results)


# revision 12
# speedup vs baseline: 1.0879x; 1.0879x over previous
"""Single-head causal attention on 8 TRN2 NeuronCores, data-parallel over batch.

Problem: x [512, 256, 384] f32, Wq/Wk/Wv [384, 64] f32.
  q/k/v = x @ W;  S = q k^T / sqrt(384); causal softmax; out = P v.

Sharding: batch 512 -> 64 per core.  Host pre-transposes x so each device DMA
is contiguous; weights replicated.

Device: pair-granular software pipeline built so the PE never waits: every
dependency consumed by the PE is >= 2 iterations old.
  iter i: [produce: QK(quad) on even / V(quad) on odd] PV(i-4) ST(i-2)
  - qkT [128, 4, 256] = [Wq*scale | Wk] stationary over xT chunks (quad, even
    iters).  One DVE copy to SBUF bf16; k halves move to base-0 pair tiles by
    gpsimd DIRECT2D copies (one per iteration, queued after the mask select).
  - vT computed ALREADY transposed ([t, h]) with the xT chunks stationary
    (quad, odd iters); ones column seeded once per buffer slot for rowsums.
  - ST per pair in 3 blocks of 128 cols: [tri(s0,t0) | tri(s1,t1) |
    full(s0,t1)]; s>t blocks skipped.  One exp per pair ([128, 768], scalar),
    one 3D affine_select per pair zeroes both triangles (gpsimd, first in its
    queue each iteration).
  - PV (4 iterations behind) accumulates [t, 64+rowsum]; o_ps copied to a
    quad SBUF tile in bf16 (DVE for pair a, scalar for pair b) and DMA'd out
    UNNORMALIZED; the softmax division happens on the host in f32.
"""

import numpy as np

import concourse.bacc as bacc
import concourse.bass as bass
import concourse.mybir as mybir
import concourse.tile as tile
from concourse.bass_utils import run_bass_kernel_spmd

N_CORES = 8
B, T, C, H = 512, 256, 384, 64
BPC = B // N_CORES          # 64 batches per core
PAIRS = BPC // 2            # 32 pair-iterations per core
QUADS = BPC // 4            # 16 quad-DMA groups
NCHUNK = C // 128           # 3 contraction chunks
SCALE = 1.0 / np.sqrt(C)    # note: reference scales by C**-0.5, not H**-0.5

F32 = mybir.dt.float32
BF16 = mybir.dt.bfloat16
EXP = mybir.ActivationFunctionType.Exp


def build_bass():
    nc = bacc.Bacc(None, target_bir_lowering=False, debug=False)
    x_in = nc.dram_tensor("xt", [QUADS, 128, NCHUNK, 4, T], BF16, kind="ExternalInput")
    wqk_in = nc.dram_tensor("wqk", [128, NCHUNK, 128], BF16, kind="ExternalInput")
    wv_in = nc.dram_tensor("wv", [128, NCHUNK, H], BF16, kind="ExternalInput")
    out_d = nc.dram_tensor("out", [QUADS, 128, 4, 2, H + 1], BF16, kind="ExternalOutput")

    with tile.TileContext(nc) as tc:
        with (
            tc.tile_pool(name="const", bufs=1) as const_pool,
            tc.tile_pool(name="xt", bufs=3) as xt_pool,
            tc.tile_pool(name="qk_sb", bufs=3) as qk_pool,
            tc.tile_pool(name="k_sb", bufs=4) as k_pool,
            tc.tile_pool(name="v_sb", bufs=4) as v_pool,
            tc.tile_pool(name="p_sb", bufs=3) as p_pool,
            tc.tile_pool(name="ob", bufs=2) as ob_pool,
            tc.tile_pool(name="qk_ps", bufs=1, space="PSUM") as qk_ps_pool,
            tc.tile_pool(name="v_ps", bufs=2, space="PSUM") as v_ps_pool,
            tc.tile_pool(name="st_ps", bufs=1, space="PSUM") as st_ps_pool,
            tc.tile_pool(name="o_ps", bufs=2, space="PSUM") as o_ps_pool,
        ):
            wqk = const_pool.tile([128, NCHUNK, 128], BF16)
            nc.sync.dma_start(wqk[:], wqk_in[:])
            wv = const_pool.tile([128, NCHUNK, H], BF16)
            nc.sync.dma_start(wv[:], wv_in[:])

            xts, qks, ks, vs, pss, obs = {}, {}, {}, {}, {}, {}

            for i in range(PAIRS + 4):
                # ---- produce: QK(quad) on even iters, V(quad) on odd -------
                if i % 2 == 0 and i // 2 < QUADS:
                    q = i // 2
                    if i == 0:
                        for pf in range(min(3, QUADS)):
                            xts[pf] = xt_pool.tile(
                                [128, NCHUNK, 4, T], BF16, tag="xt", name="xt"
                            )
                            nc.sync.dma_start(xts[pf][:], x_in[pf])
                    elif q + 2 < QUADS:
                        xts[q + 2] = xt_pool.tile(
                            [128, NCHUNK, 4, T], BF16, tag="xt", name="xt"
                        )
                        nc.sync.dma_start(xts[q + 2][:], x_in[q + 2])

                    qk_ps = qk_ps_pool.tile([128, 4, T], F32, tag="qk")
                    qks[q] = qk_pool.tile([128, 4, T], BF16, tag="qk", name="qk_sb")
                    for h in range(2):
                        for n in range(NCHUNK):
                            nc.tensor.matmul(
                                qk_ps[:, 2 * h : 2 * h + 2, :],
                                wqk[:, n, :],
                                xts[q][:, n, 2 * h : 2 * h + 2, :],
                                start=(n == 0),
                                stop=(n == NCHUNK - 1),
                            )
                        # per-half copy releases this half's PSUM region early
                        nc.vector.tensor_copy(
                            qks[q][:, 2 * h : 2 * h + 2, :],
                            qk_ps[:, 2 * h : 2 * h + 2, :],
                        )
                        # k half down to a base-0 pair tile (gpsimd DIRECT2D;
                        # ST consumes it 2 iterations from now)
                        ks[2 * q + h] = k_pool.tile(
                            [H, 2, T], BF16, tag="k", name="k_sb"
                        )
                        nc.gpsimd.dma_start(
                            ks[2 * q + h][:], qks[q][H:128, 2 * h : 2 * h + 2]
                        )
                elif i % 2 == 1 and i // 2 < QUADS:
                    q = i // 2
                    # v, already transposed to [t, h]: xT chunk is stationary
                    v_ps = v_ps_pool.tile([128, 4, 2, H], F32, tag="v")
                    for jj in range(4):
                        for tb in range(2):
                            for n in range(NCHUNK):
                                nc.tensor.matmul(
                                    v_ps[:, jj, tb, :],
                                    xts[q][:, n, jj, bass.ts(tb, 128)],
                                    wv[:, n, :],
                                    start=(n == 0),
                                    stop=(n == NCHUNK - 1),
                                )
                    vs[q] = v_pool.tile([128, 4, 2, H + 1], BF16, tag="v", name="v_sb")
                    if q < 4:
                        # ones column for the PV rowsum: each pool slot keeps
                        # it forever (the copy below never touches col H)
                        nc.gpsimd.memset(vs[q][:, :, :, H : H + 1], 1.0)
                    nc.vector.tensor_copy(vs[q][:, :, :, 0:H], v_ps[:])
                    del xts[q]

                # ---- PV(i-4) + writeback (mask select is 2 iterations old) -
                w = i - 4
                if 0 <= w < PAIRS:
                    qw, hw = divmod(w, 2)
                    o_ps = o_ps_pool.tile([128, 2, 2, H + 1], F32, tag="o")
                    for jl in range(2):
                        p = pss[w]
                        v = vs[qw]
                        jj = 2 * hw + jl
                        nc.tensor.matmul(
                            o_ps[:, jl, 0, :], p[:, jl, 0, :], v[:, jj, 0, :],
                            start=True, stop=True,
                        )
                        nc.tensor.matmul(
                            o_ps[:, jl, 1, :], p[:, jl, 2, :], v[:, jj, 0, :],
                            start=True, stop=False,
                        )
                        nc.tensor.matmul(
                            o_ps[:, jl, 1, :], p[:, jl, 1, :], v[:, jj, 1, :],
                            start=False, stop=True,
                        )
                    if hw == 0:
                        obs[qw] = ob_pool.tile(
                            [128, 4, 2, H + 1], BF16, tag="ob", name="ob"
                        )
                        nc.vector.tensor_copy(obs[qw][:, 0:2], o_ps[:])
                    else:
                        nc.scalar.copy(obs[qw][:, 2:4], o_ps[:])
                        nc.sync.dma_start(out_d[qw], obs[qw][:])
                        del obs[qw], vs[qw]
                    del pss[w]

                # ---- ST(i-2) + exp + causal mask ---------------------------
                m = i - 2
                if 0 <= m < PAIRS:
                    qm, hm = divmod(m, 2)
                    st = st_ps_pool.tile([128, 2, 3, 128], F32, tag="st")
                    for jl in range(2):
                        qT = qks[qm][0:H, 2 * hm + jl]   # [64, 256]
                        kT = ks[m][:, jl]                # [64, 256]
                        nc.tensor.matmul(
                            st[:, jl, 0, :], kT[:, 0:128], qT[:, 0:128],
                            start=True, stop=True,
                        )
                        nc.tensor.matmul(
                            st[:, jl, 2, :], kT[:, 0:128], qT[:, 128:T],
                            start=True, stop=True,
                        )
                        nc.tensor.matmul(
                            st[:, jl, 1, :], kT[:, 128:T], qT[:, 128:T],
                            start=True, stop=True,
                        )
                    pss[m] = p_pool.tile([128, 2, 3, 128], BF16, tag="p", name="p_sb")
                    nc.scalar.activation(pss[m][:], st[:], EXP)
                    # zero s > t in both triangular blocks of both batches:
                    # keep where col - partition >= 0
                    nc.gpsimd.affine_select(
                        out=pss[m][:, :, 0:2, :],
                        in_=pss[m][:, :, 0:2, :],
                        compare_op=mybir.AluOpType.is_ge,
                        fill=0.0,
                        base=0,
                        pattern=[[0, 2], [0, 2], [1, 128]],
                        channel_multiplier=-1,
                    )
                    del ks[m]
                    if hm == 1:
                        del qks[qm]



    nc.finalize()
    return nc


_CACHED = {}


def _get_nc():
    if "nc" not in _CACHED:
        _CACHED["nc"] = build_bass()
    return _CACHED["nc"]


def prep_inputs(x, Wq, Wk, Wv):
    import ml_dtypes

    bf16 = ml_dtypes.bfloat16
    x = np.ascontiguousarray(x, dtype=np.float32)
    wqk = np.concatenate([Wq * SCALE, Wk], axis=1).astype(np.float32)  # [384, 128]
    wqk_t = np.ascontiguousarray(
        wqk.reshape(NCHUNK, 128, 128).transpose(1, 0, 2).astype(bf16)
    )
    wv_t = np.ascontiguousarray(
        Wv.astype(np.float32).reshape(NCHUNK, 128, H).transpose(1, 0, 2).astype(bf16)
    )

    in_maps = []
    for c in range(N_CORES):
        xs = x[c * BPC : (c + 1) * BPC]  # [64, 256, 384]
        # [q, jj, t, n, p] -> [q, p, n, jj, t]  (partition-major for the DMA)
        xt = np.ascontiguousarray(
            xs.reshape(QUADS, 4, T, NCHUNK, 128).transpose(0, 4, 3, 1, 2).astype(bf16)
        )
        in_maps.append({"xt": xt, "wqk": wqk_t, "wv": wv_t})
    return in_maps


def postprocess(results):
    outs = []
    for c in range(N_CORES):
        od = results[c]["out"].astype(np.float32)  # [QUADS, 128p, 4jj, 2n, H+1]
        o = od[..., 0:H] / od[..., H : H + 1]
        outs.append(o.transpose(0, 2, 3, 1, 4).reshape(BPC, T, H))
    return np.concatenate(outs, axis=0).astype(np.float32)


def kernel(x, Wq, Wk, Wv):
    in_maps = prep_inputs(x, Wq, Wk, Wv)
    res = run_bass_kernel_spmd(_get_nc(), in_maps, core_ids=list(range(N_CORES)))
    return postprocess(res.results)
